# revision 4
# baseline (speedup 1.0000x reference)
"""Trainium2 Bass kernel for the kNN pairwise-ranking loss.

Math: with y = (knn_tgts == tgts), the masked pairwise BCE-with-logits loss
over differing-label pairs (j > i) collapses to

    loss = sum_b sum_{n in neg_b} sum_{p in pos_b} softplus(s_n - s_p) / cnt
    cnt  = sum_b |pos_b| * |neg_b|

because for a (pos, neg) pair the per-pair term is softplus(s_neg - s_pos)
regardless of orientation, and b2 cancels in score differences.

Host side: per batch row, permute keys so positives come first, then
negatives.  Rows are sorted by positive count and dealt to (core, slot) so
each slot's compile-time widths are tight.  Additive kill values (+-200 on
invalid lanes) make killed exponentials underflow to exactly 0.

Device (SPMD over 8 cores, 4 batch rows each):
  phase A (per row): h = relu(W1 @ keys^T + b1) via PE (fp8 DoubleRow, f32
                     psum).  hh is augmented with two DMA'd pad rows
                     (npad at p100, ppad at p101).
  phase B (per row): ONE score matmul stripe per sign into S row 0:
                     cols [0:Pwt) hold -(s_p + ppad) (lhsT -w2a col), cols
                     [Pwt : Pwt+128*nch) hold s_n + npad; ONE Exp pass
                     produces the combined strip T [1, Pwt+128*nch];
                     gpsimd partition_broadcast replicates T to all 128
                     partitions (Bc); rank-1 outer products run 3-way
                     row-packed on the PE (tile q = chunk%3 at quadrant
                     32q writes only PSUM bank q of the op tile) to form
                     e^{s_n - s_p}; Ln(1+x) with accum_out reduces each
                     op tile in one 4D-AP ACT pass.
  Keys DMA descriptors are posted in parallel on sync (keys) and scalar
  (consts) sequencers right at main start; per-row partial accumulator
  columns are DMA'd out on the scalar queue as soon as they are final.
Host gathers [128, ncalls] partial sums, reduces, divides by cnt.
"""

import numpy as np

B, K, D, H = 32, 1024, 1024, 100
N_CORES = 8
BPC = B // N_CORES  # batch rows per core
KILL = 200.0

_cache = {}
_act_patched = False


def _patch_act_tables():
    """Make Exp/Ln resolve to the single combined ACT table set."""
    global _act_patched
    if _act_patched:
        return
    import concourse.bacc as bacc
    import concourse.hw_specs as hw_specs
    import concourse.mybir as mybir

    orig = hw_specs.get_activation_tables
    combined = "natural_log_exp_and_others"

    def patched(arch):
        tabs = orig(arch)
        out = {}
        for name, funcs in tabs.items():
            f = set(funcs)
            if name != combined and combined in tabs:
                f.discard(mybir.ActivationFunctionType.Exp)
                f.discard(mybir.ActivationFunctionType.Ln)
            out[name] = f
        return out

    hw_specs.get_activation_tables = patched
    bacc.get_activation_tables = patched
    _act_patched = True


def _slot_geometry(Pw, nch):
    """Chunk geometry for one slot."""
    nps = 3
    cw = -(-Pw // nps)  # ceil
    Pwt = nps * cw  # padded positive width (<= 512 required)
    assert Pwt <= 512 and Pwt + 128 * nch <= 1408
    nchk = nch * nps  # total rank-1 chunks, multiple of 3
    spb = 512 // cw  # chunk slots per PSUM bank
    cap = 3 * spb  # chunks per op tile (tile q -> bank q)
    groups = []
    c0 = 0
    while c0 < nchk:
        n = min(cap, nchk - c0)
        groups.append(n // 3)  # slots-per-bank m for this group
        c0 += n
    return cw, Pwt, nchk, groups


def _build_program(slot_params):
    """slot_params: tuple of (Pw, nch) per slot; nst = 1024 - 128*nch."""
    import concourse.bacc as bacc
    import concourse.mybir as mybir
    import concourse.tile as tile

    _patch_act_tables()

    f32 = mybir.dt.float32
    bf16 = mybir.dt.bfloat16
    fp8 = mybir.dt.float8e4
    ndc = 4  # contraction chunks (256 wide with DoubleRow)
    hpad = 112  # padded per-subrow weight stride (DoubleRow needs step%16==0)
    wchunk = 2 * hpad

    geo = [_slot_geometry(Pw, nch) for Pw, nch in slot_params]
    ncalls = sum(len(g[3]) for g in geo)

    nc = bacc.Bacc(
        "TRN2",
        target_bir_lowering=False,
        debug=False,
        enable_asserts=False,
        num_devices=N_CORES,
    )

    keys_d = nc.dram_tensor(
        "keys_t", [BPC, 128, ndc * 2 * K], fp8, kind="ExternalInput"
    ).ap()
    w1t_d = nc.dram_tensor("w1t", [128, ndc * wchunk], fp8, kind="ExternalInput").ap()
    w2a_d = nc.dram_tensor("w2a", [H + 2, 2], bf16, kind="ExternalInput").ap()
    b1_d = nc.dram_tensor("b1c", [H, 1], f32, kind="ExternalInput").ap()
    pads_d = nc.dram_tensor("pads", [BPC, 6, K], bf16, kind="ExternalInput").ap()
    out_d = nc.dram_tensor("acc_out", [128, ncalls], f32, kind="ExternalOutput").ap()

    with tile.TileContext(nc) as tc:
        with (
            tc.tile_pool(name="const", bufs=1) as cpool,
            tc.tile_pool(name="keys", bufs=2 * BPC + 2) as kpool,
            tc.tile_pool(name="h", bufs=BPC) as hpool,
            tc.tile_pool(name="T1", bufs=2) as t1pool,
            tc.tile_pool(name="bc", bufs=2) as bcpool,
            tc.tile_pool(name="hp", bufs=2, space="PSUM") as hp_pool,
            tc.tile_pool(name="op", bufs=2, space="PSUM") as op_pool,
        ):
            acc_sb = cpool.tile([128, ncalls], f32, tag="acc")
            dummy_sb = cpool.tile([128, 3, 512], bf16, tag="dummy")
            tmp_sb = cpool.tile([1, 2], f32, tag="tmp")

            # trigger the ACT table load as the first scalar-engine work
            nc.vector.memset(tmp_sb[:], 0.0)
            nc.scalar.activation(
                tmp_sb[0:1, 1:2], tmp_sb[0:1, 0:1],
                mybir.ActivationFunctionType.Exp, scale=1.0,
            )

            # ---- DMA issue: consts on scalar, keys on sync, pads gpsimd ----
            w1t_sb = cpool.tile([128, ndc * wchunk], fp8, tag="w1t")
            nc.scalar.dma_start(w1t_sb[:], w1t_d[:])
            w2a_sb = cpool.tile([H + 2, 2], bf16, tag="w2a")
            nc.scalar.dma_start(w2a_sb[:], w2a_d[:])
            b1_sb = cpool.tile([H, 1], f32, tag="b1")
            nc.scalar.dma_start(b1_sb[:], b1_d[:])

            # row-0 keys as quarter-DMAs so the first matmul starts sooner;
            # rows 1-3 as halves, all on the sync queue in consumption order
            kq = []
            quads = []
            for i in range(4):
                kt = kpool.tile([128, 2 * K], fp8, tag="keys")
                nc.sync.dma_start(kt[:], keys_d[0, :, i * 2 * K : (i + 1) * 2 * K])
                quads.append((kt, 0))
            kq.append(quads)
            for r in range(1, BPC):
                quads = []
                for hf in range(2):
                    kt = kpool.tile([128, ndc * K], fp8, tag="keys")
                    nc.sync.dma_start(
                        kt[:], keys_d[r, :, hf * ndc * K : (hf + 1) * ndc * K]
                    )
                    quads += [(kt, 0), (kt, 2 * K)]
                kq.append(quads)

            # hh tiles: pad rows ride at partitions 100:102 via gpsimd SWDGE
            hhs = []
            for r in range(BPC):
                hh = hpool.tile([H + 2, K], bf16, tag="h")
                nc.gpsimd.dma_start(hh[96 : H + 2, :], pads_d[r, :, :])
                hhs.append(hh)

            scs = [None] * BPC
            Ts = [None] * BPC
            Bcs = [None] * BPC
            state = {"call": 0}

            def stageAmm(r):
                hh = hhs[r]
                hp0 = hp_pool.tile([H, 512], f32, tag="hpx")
                hp1 = hp_pool.tile([H, 512], f32, tag="hpx")
                hps = [hp0, hp1]
                for dc in range(ndc):
                    kt, c0 = kq[r][dc]
                    kt3 = kt[:, c0 : c0 + 2 * K].rearrange("p (i k) -> p i k", i=2)
                    w_sl = w1t_sb[:, dc * wchunk : (dc + 1) * wchunk].rearrange(
                        "p (i m) -> p i m", i=2
                    )[:, :, 0:H]
                    for kh in range(2):
                        nc.tensor.matmul(
                            hps[kh][:, :],
                            lhsT=w_sl,
                            rhs=kt3[:, :, kh * 512 : (kh + 1) * 512],
                            start=(dc == 0),
                            stop=(dc == ndc - 1),
                            perf_mode=mybir.MatmulPerfMode.DoubleRow,
                        )
                for kh in range(2):
                    nc.vector.tensor_scalar(
                        hh[0:H, kh * 512 : (kh + 1) * 512],
                        hps[kh][:, :],
                        b1_sb[:],
                        0.0,
                        op0=mybir.AluOpType.add,
                        op1=mybir.AluOpType.max,
                    )

            def stageScore(r):
                """Scores into S row 0: [0:Pwt) = -(s_p+ppad), then
                [Pwt : Pwt+128*nch) = s_n + npad for keys nst:K."""
                Pw, nch = slot_params[r]
                cw, Pwt, nchk, groups = geo[r]
                nst = K - 128 * nch
                hh = hhs[r]
                S = op_pool.tile([64, 3 * 512], f32, tag="op")
                scs[r] = S
                # pos: needs only the kh0 half of relu (Pwt <= 512)
                nc.tensor.matmul(
                    S[0:1, 0:Pwt],
                    lhsT=w2a_sb[:, 1:2],
                    rhs=hh[:, 0:Pwt],
                    start=True,
                    stop=True,
                )
                # neg: hh cols nst:K -> S cols Pwt:Pwt+128*nch, split on
                # both hh kh halves (512) and S psum bank boundaries (512)
                segs = []
                c = nst
                while c < K:
                    hh_end = 512 if c < 512 else K
                    s_col = Pwt + (c - nst)
                    s_end = c + (512 - s_col % 512) if (s_col % 512) else c + 512
                    e = min(hh_end, s_end, K)
                    segs.append((c, e))
                    c = e
                for c, e in segs:
                    nc.tensor.matmul(
                        S[0:1, Pwt + (c - nst) : Pwt + (e - nst)],
                        lhsT=w2a_sb[:, 0:1],
                        rhs=hh[:, c:e],
                        start=True,
                        stop=True,
                    )

            def stageExp(r):
                Pw, nch = slot_params[r]
                cw, Pwt, nchk, groups = geo[r]
                L = Pwt + 128 * nch
                T = t1pool.tile([1, 1408], bf16, tag="T1")
                Ts[r] = T
                nc.scalar.activation(
                    T[0:1, 0:L],
                    scs[r][0:1, 0:L],
                    mybir.ActivationFunctionType.Exp,
                    scale=1.0,
                )

            def stageBcast(r):
                cw, Pwt, nchk, groups = geo[r]
                L = Pwt + 128 * slot_params[r][1]
                Bc = bcpool.tile([128, 1408], bf16, tag="bc")
                Bcs[r] = Bc
                nc.gpsimd.partition_broadcast(Bc[:, 0:L], Ts[r][0:1, 0:L])

            def stageOuterLn(r, gi):
                """Emit one op tile's chunks (3*m rank-1 matmuls) + 1 Ln."""
                Pw, nch = slot_params[r]
                cw, Pwt, nchk, groups = geo[r]
                m = groups[gi]
                c0 = sum(3 * groups[k] for k in range(gi))
                Bc = Bcs[r]
                op = op_pool.tile([128, 3, 512], f32, tag="op")
                for i in range(3 * m):
                    c = c0 + i
                    q = c % 3
                    j = c // 3  # neg 128-block
                    h = q  # pos third handled by tile q
                    # within-tile slot: how many chunks with this q so far
                    s = i // 3
                    nc.tensor.matmul(
                        op[:, q : q + 1, s * cw : (s + 1) * cw],
                        lhsT=Bc[32 * q : 32 * q + 1, Pwt + j * 128 : Pwt + (j + 1) * 128],
                        rhs=Bc[32 * q : 32 * q + 1, h * cw : (h + 1) * cw],
                        start=True,
                        stop=True,
                        tile_position=(32 * q, 0),
                    )
                src = op[:, :, 0 : m * cw].rearrange("p b (i w) -> p b i w", i=m)
                dst = dummy_sb[:, :, 0 : m * cw].rearrange("p b (i w) -> p b i w", i=m)
                nc.scalar.activation(
                    dst,
                    src,
                    mybir.ActivationFunctionType.Ln,
                    bias=1.0,
                    scale=1.0,
                    accum_out=acc_sb[:, state["call"] : state["call"] + 1],
                )
                state["call"] += 1

            # chunk-to-block mapping sanity: chunk c covers neg block c//3
            # paired with pos third c%3; over c = 0..3*nch-1 this hits every
            # (j, h) exactly once.

            call_lo = 0
            stageAmm(0)
            stageAmm(1)
            stageScore(0)
            stageExp(0)
            stageBcast(0)
            for r in range(BPC):
                ngroups = len(geo[r][3])
                stageOuterLn(r, 0)
                if r + 2 < BPC:
                    stageAmm(r + 2)
                if r + 1 < BPC:
                    stageScore(r + 1)
                    stageExp(r + 1)
                    stageBcast(r + 1)
                for gi in range(1, ngroups):
                    stageOuterLn(r, gi)
                # ship this row's accumulator columns on the scalar queue
                nc.scalar.dma_start(
                    out_d[:, call_lo : state["call"]],
                    acc_sb[:, call_lo : state["call"]],
                )
                call_lo = state["call"]

    nc.compile()
    return nc, state["call"]


def kernel(keys, tgts, knn_tgts, mask, W1, b1, W2, b2, _profile=False):
    import ml_dtypes

    from concourse.bass_utils import run_bass_kernel_spmd

    keys = np.asarray(keys, dtype=np.float32)
    tgts = np.asarray(tgts)
    knn_tgts = np.asarray(knn_tgts)
    mask = np.asarray(mask).astype(bool)
    W1 = np.asarray(W1, dtype=np.float32)
    b1 = np.asarray(b1, dtype=np.float32)
    W2 = np.asarray(W2, dtype=np.float32)

    # ---- host-side label/permutation prep ----
    y = knn_tgts == tgts[:, None]
    pos = y & mask
    neg = (~y) & mask
    P = pos.sum(axis=1).astype(np.int64)
    N_ = neg.sum(axis=1).astype(np.int64)
    cnt = float((P * N_).sum())

    # stable order: positives, negatives, masked-out
    rank = np.where(pos, 0, np.where(neg, 1, 2)).astype(np.int8)
    order = np.argsort(rank, axis=1, kind="stable")  # [B, K]

    # deal rows sorted by P desc: rank i -> core i%8, slot i//8
    rows_by_p = np.argsort(-P, kind="stable")
    assign = rows_by_p.reshape(BPC, N_CORES)  # [slot, core] -> row id

    slot_params = []
    for r in range(BPC):
        ps = P[assign[r]]
        Pw = int(ps.max())
        nch = (K - int(ps.min()) + 127) // 128
        slot_params.append((Pw, nch))
    slot_params = tuple(slot_params)

    # permuted, transposed keys in fp8 DoubleRow layout: per row [128, 8K]
    keys_perm = np.take_along_axis(keys, order[:, :, None], axis=1)  # [B, K, D]
    kt = keys_perm.transpose(0, 2, 1).astype(ml_dtypes.float8_e4m3)  # [B, D, K]
    kt = np.ascontiguousarray(
        kt.reshape(B, 4, 2, 128, K).transpose(0, 3, 1, 2, 4).reshape(B, 128, 8 * K)
    )

    # scale W1 by 16 into fp8's sweet spot; fold 1/16 into W2 and 16 into b1
    hpad = 112
    ndc = 4
    w1s = (W1.T * 16.0).astype(np.float32)  # [D, H]
    w4 = np.zeros((ndc, 2, 128, hpad), dtype=np.float32)
    w4[:, :, :, :H] = w1s.reshape(ndc, 2, 128, H)
    w1t = np.ascontiguousarray(
        w4.transpose(2, 0, 1, 3).reshape(128, ndc * 2 * hpad)
    ).astype(ml_dtypes.float8_e4m3)
    w2v = (W2.reshape(H) / 16.0).astype(np.float32)
    w2a = np.zeros((H + 2, 2), dtype=np.float32)
    w2a[:H, 0] = w2v
    w2a[H, 0] = 1.0  # + npad row
    w2a[:H, 1] = -w2v
    w2a[H + 1, 1] = -1.0  # - ppad row
    w2a = w2a.astype(ml_dtypes.bfloat16)
    b1c = np.ascontiguousarray(b1.reshape(H, 1) * 16.0)

    # pad rows riding in hh partitions 100:102 (rows 4:6 of a 6-row block
    # whose rows 0:4 land on junk partitions 96:100):
    #   npad[j] = -KILL for j < P  (kills positives on the negative side)
    #   ppad[j] = +KILL for j >= P (kills non-positives on the positive side)
    kidx = np.arange(K)[None, :]
    pads = np.zeros((N_CORES, BPC, 6, K), dtype=np.float32)
    for r in range(BPC):
        pr = P[assign[r]][:, None]  # [cores, 1]
        pads[:, r, 4, :] = np.where(kidx < pr, -KILL, 0.0)
        pads[:, r, 5, :] = np.where(kidx < pr, 0.0, KILL)
    pads = pads.astype(ml_dtypes.bfloat16)

    key = slot_params
    if key not in _cache:
        _cache[key] = _build_program(slot_params)
    nc, ncalls = _cache[key]

    in_maps = []
    for c in range(N_CORES):
        in_maps.append(
            {
                "keys_t": np.ascontiguousarray(kt[assign[:, c]]),
                "w1t": w1t,
                "w2a": w2a,
                "b1c": b1c,
                "pads": np.ascontiguousarray(pads[c]),
            }
        )

    res = run_bass_kernel_spmd(
        nc, in_maps, list(range(N_CORES)), trace=bool(_profile)
    )
    total = 0.0
    for r in res.results:
        total += float(r["acc_out"].astype(np.float64).sum())
    if _profile:
        print(f"HW exec time: {res.exec_time_ns} ns")
        globals()["_last_results"] = res
    loss = np.float64(total) / np.float64(cnt)
    return np.array(loss, dtype=np.float32)


# revision 17
# speedup vs baseline: 1.0486x; 1.0486x over previous
"""Trainium2 Bass kernel for the kNN pairwise-ranking loss.

Math: with y = (knn_tgts == tgts), the masked pairwise BCE-with-logits loss
over differing-label pairs (j > i) collapses to

    loss = sum_b sum_{n in neg_b} sum_{p in pos_b} softplus(s_n - s_p) / cnt
    cnt  = sum_b |pos_b| * |neg_b|

because for a (pos, neg) pair the per-pair term is softplus(s_neg - s_pos)
regardless of orientation, and b2 cancels in score differences.

Host side: per batch row, permute keys so positives come first, then
negatives.  Rows are sorted by positive count and dealt to (core, slot) so
each slot's compile-time widths are tight.  Additive kill values (+-200 on
invalid lanes) make killed exponentials underflow to exactly 0.

Device (SPMD over 8 cores, 4 batch rows each):
  phase A (per row): h = relu(W1 @ keys^T + b1) via PE (fp8 DoubleRow, f32
                     psum).  hh is augmented with two DMA'd pad rows
                     (npad at p100, ppad at p101).
  phase B (per row): ONE score matmul stripe per sign into S row 0:
                     cols [0:Pwt) hold -(s_p + ppad) (lhsT -w2a col), cols
                     [Pwt : Pwt+128*nch) hold s_n + npad; ONE Exp pass
                     produces the combined strip T [1, Pwt+128*nch];
                     gpsimd partition_broadcast replicates T to all 128
                     partitions (Bc); rank-1 outer products run 3-way
                     row-packed on the PE (tile q = chunk%3 at quadrant
                     32q writes only PSUM bank q of the op tile) to form
                     e^{s_n - s_p}; Ln(1+x) with accum_out reduces each
                     op tile in one 4D-AP ACT pass.
  Keys DMA descriptors are posted in parallel on sync (keys) and scalar
  (consts) sequencers right at main start; per-row partial accumulator
  columns are DMA'd out on the scalar queue as soon as they are final.
Host gathers [128, ncalls] partial sums, reduces, divides by cnt.
"""

import numpy as np

B, K, D, H = 32, 1024, 1024, 100
N_CORES = 8
BPC = B // N_CORES  # batch rows per core
KILL = 200.0

_cache = {}
_act_patched = False


def _patch_act_tables():
    """Make Exp/Ln resolve to the single combined ACT table set."""
    global _act_patched
    if _act_patched:
        return
    import concourse.bacc as bacc
    import concourse.hw_specs as hw_specs
    import concourse.mybir as mybir

    orig = hw_specs.get_activation_tables
    combined = "natural_log_exp_and_others"

    def patched(arch):
        tabs = orig(arch)
        out = {}
        for name, funcs in tabs.items():
            f = set(funcs)
            if name != combined and combined in tabs:
                f.discard(mybir.ActivationFunctionType.Exp)
                f.discard(mybir.ActivationFunctionType.Ln)
            out[name] = f
        return out

    hw_specs.get_activation_tables = patched
    bacc.get_activation_tables = patched
    _act_patched = True


def _slot_geometry(Pw, nch, last):
    """Chunk geometry for one slot.

    Rows 0..BPC-2 run 3-way row-packed (nps=3, tile q = chunk%3 at
    quadrant 32q, writing only PSUM bank q).  The last row skips the
    partition broadcast (drain latency) and runs unpacked (nps=1) with
    the same chunk->bank striping on the full-array tile.
    """
    nps = 1 if last else 3
    cw = -(-Pw // nps)  # ceil
    Pwt = nps * cw  # padded positive width (<= 512 required)
    assert Pwt <= 512 and Pwt + 128 * nch <= 1408
    nchk = nch * nps  # total rank-1 chunks
    spb = 512 // cw  # chunk slots per PSUM bank
    cap = 3 * spb  # chunks per op tile (chunk c -> bank c%3, slot c//3)
    groups = []  # (c0, n) per LN call; n % 3 == 0 or n < 3
    c0 = 0
    while c0 < nchk:
        n = min(cap, nchk - c0)
        groups.append((c0, n))
        c0 += n
    return nps, cw, Pwt, nchk, groups


def _build_program(slot_params):
    """slot_params: tuple of (Pw, nch) per slot; nst = 1024 - 128*nch."""
    import concourse.bacc as bacc
    import concourse.mybir as mybir
    import concourse.tile as tile

    _patch_act_tables()

    f32 = mybir.dt.float32
    bf16 = mybir.dt.bfloat16
    fp8 = mybir.dt.float8e4
    ndc = 4  # contraction chunks (256 wide with DoubleRow)
    hpad = 112  # padded per-subrow weight stride (DoubleRow needs step%16==0)
    wchunk = 2 * hpad

    geo = [
        _slot_geometry(Pw, nch, r == len(slot_params) - 1)
        for r, (Pw, nch) in enumerate(slot_params)
    ]
    ncalls = sum(len(g[4]) for g in geo)

    nc = bacc.Bacc(
        "TRN2",
        target_bir_lowering=False,
        debug=False,
        enable_asserts=False,
        num_devices=N_CORES,
    )

    keys_d = nc.dram_tensor(
        "keys_t", [BPC, 128, ndc * 2 * K], fp8, kind="ExternalInput"
    ).ap()
    w1t_d = nc.dram_tensor("w1t", [128, ndc * wchunk], fp8, kind="ExternalInput").ap()
    w2a_d = nc.dram_tensor("w2a", [H + 2, 2], bf16, kind="ExternalInput").ap()
    b1_d = nc.dram_tensor("b1c", [H, 1], f32, kind="ExternalInput").ap()
    pads_d = nc.dram_tensor("pads", [BPC, 6, K], bf16, kind="ExternalInput").ap()
    out_d = nc.dram_tensor("acc_out", [128, ncalls], f32, kind="ExternalOutput").ap()

    with tile.TileContext(nc) as tc:
        with (
            tc.tile_pool(name="const", bufs=1) as cpool,
            tc.tile_pool(name="keys", bufs=2 * BPC + 2) as kpool,
            tc.tile_pool(name="h", bufs=BPC) as hpool,
            tc.tile_pool(name="T1", bufs=2) as t1pool,
            tc.tile_pool(name="bc", bufs=2) as bcpool,
            tc.tile_pool(name="hp", bufs=2, space="PSUM") as hp_pool,
            tc.tile_pool(name="op", bufs=2, space="PSUM") as op_pool,
        ):
            acc_sb = cpool.tile([128, ncalls], f32, tag="acc")
            dummy_sb = cpool.tile([128, 3, 512], bf16, tag="dummy")
            tmp_sb = cpool.tile([1, 2], f32, tag="tmp")

            # trigger the ACT table load as the first scalar-engine work
            nc.vector.memset(tmp_sb[:], 0.0)
            nc.scalar.activation(
                tmp_sb[0:1, 1:2], tmp_sb[0:1, 0:1],
                mybir.ActivationFunctionType.Exp, scale=1.0,
            )

            # ---- DMA issue: consts on scalar, keys on sync, pads gpsimd ----
            w1t_sb = cpool.tile([128, ndc * wchunk], fp8, tag="w1t")
            nc.scalar.dma_start(w1t_sb[:], w1t_d[:])
            w2a_sb = cpool.tile([H + 2, 2], bf16, tag="w2a")
            nc.scalar.dma_start(w2a_sb[:], w2a_d[:])
            b1_sb = cpool.tile([H, 1], f32, tag="b1")
            nc.scalar.dma_start(b1_sb[:], b1_d[:])

            # row-0 keys as quarter-DMAs so the first matmul starts sooner;
            # rows 1-3 as halves, all on the sync queue in consumption order
            kq = []
            quads = []
            for i in range(4):
                kt = kpool.tile([128, 2 * K], fp8, tag="keys")
                nc.sync.dma_start(kt[:], keys_d[0, :, i * 2 * K : (i + 1) * 2 * K])
                quads.append((kt, 0))
            kq.append(quads)
            for r in range(1, BPC):
                quads = []
                for hf in range(2):
                    kt = kpool.tile([128, ndc * K], fp8, tag="keys")
                    nc.sync.dma_start(
                        kt[:], keys_d[r, :, hf * ndc * K : (hf + 1) * ndc * K]
                    )
                    quads += [(kt, 0), (kt, 2 * K)]
                kq.append(quads)

            # hh tiles: pad rows ride at partitions 100:102 via gpsimd SWDGE
            hhs = []
            for r in range(BPC):
                hh = hpool.tile([H + 2, K], bf16, tag="h")
                nc.gpsimd.dma_start(hh[96 : H + 2, :], pads_d[r, :, :])
                hhs.append(hh)

            scs = [None] * BPC
            Ts = [None] * BPC
            Bcs = [None] * BPC
            state = {"call": 0}

            def stageAmm(r):
                hh = hhs[r]
                hp0 = hp_pool.tile([H, 512], f32, tag="hpx")
                hp1 = hp_pool.tile([H, 512], f32, tag="hpx")
                hps = [hp0, hp1]
                for dc in range(ndc):
                    kt, c0 = kq[r][dc]
                    kt3 = kt[:, c0 : c0 + 2 * K].rearrange("p (i k) -> p i k", i=2)
                    w_sl = w1t_sb[:, dc * wchunk : (dc + 1) * wchunk].rearrange(
                        "p (i m) -> p i m", i=2
                    )[:, :, 0:H]
                    for kh in range(2):
                        nc.tensor.matmul(
                            hps[kh][:, :],
                            lhsT=w_sl,
                            rhs=kt3[:, :, kh * 512 : (kh + 1) * 512],
                            start=(dc == 0),
                            stop=(dc == ndc - 1),
                            perf_mode=mybir.MatmulPerfMode.DoubleRow,
                        )
                for kh in range(2):
                    nc.vector.tensor_scalar(
                        hh[0:H, kh * 512 : (kh + 1) * 512],
                        hps[kh][:, :],
                        b1_sb[:],
                        0.0,
                        op0=mybir.AluOpType.add,
                        op1=mybir.AluOpType.max,
                    )

            def stageScore(r):
                """Scores into S row 0: [0:Pwt) = -(s_p+ppad), then
                [Pwt : Pwt+128*nch) = s_n + npad for keys nst:K."""
                Pw, nch = slot_params[r]
                nps, cw, Pwt, nchk, groups = geo[r]
                nst = K - 128 * nch
                hh = hhs[r]
                S = op_pool.tile([64, 3 * 512], f32, tag="op")
                scs[r] = S
                # pos: needs only the kh0 half of relu (Pwt <= 512)
                nc.tensor.matmul(
                    S[0:1, 0:Pwt],
                    lhsT=w2a_sb[:, 1:2],
                    rhs=hh[:, 0:Pwt],
                    start=True,
                    stop=True,
                )
                # neg: hh cols nst:K -> S cols Pwt:Pwt+128*nch, split on
                # both hh kh halves (512) and S psum bank boundaries (512)
                segs = []
                c = nst
                while c < K:
                    hh_end = 512 if c < 512 else K
                    s_col = Pwt + (c - nst)
                    s_end = c + (512 - s_col % 512) if (s_col % 512) else c + 512
                    e = min(hh_end, s_end, K)
                    segs.append((c, e))
                    c = e
                for c, e in segs:
                    nc.tensor.matmul(
                        S[0:1, Pwt + (c - nst) : Pwt + (e - nst)],
                        lhsT=w2a_sb[:, 0:1],
                        rhs=hh[:, c:e],
                        start=True,
                        stop=True,
                    )

            def stageExp(r):
                Pw, nch = slot_params[r]
                nps, cw, Pwt, nchk, groups = geo[r]
                L = Pwt + 128 * nch
                T = t1pool.tile([1, 1408], bf16, tag="T1")
                Ts[r] = T
                nc.scalar.activation(
                    T[0:1, 0:L],
                    scs[r][0:1, 0:L],
                    mybir.ActivationFunctionType.Exp,
                    scale=1.0,
                )

            def stageBcast(r):
                """Replicate the exp strip to all partitions so the packed
                tiles at quadrants 32/64 can read it; tile 0 reads T
                directly.  The last row runs unpacked and skips this."""
                nps, cw, Pwt, nchk, groups = geo[r]
                if nps == 1:
                    return
                L = Pwt + 128 * slot_params[r][1]
                Bc = bcpool.tile([128, 1408], bf16, tag="bc")
                Bcs[r] = Bc
                nc.gpsimd.partition_broadcast(Bc[:, 0:L], Ts[r][0:1, 0:L])

            def stageOuterLn(r, gi):
                """Emit one op tile's chunks (rank-1 matmuls) + 1 Ln."""
                Pw, nch = slot_params[r]
                nps, cw, Pwt, nchk, groups = geo[r]
                c0, n = groups[gi]
                Bc = Bcs[r]
                T = Ts[r]
                op = op_pool.tile([128, 3, 512], f32, tag="op")
                for i in range(n):
                    c = c0 + i
                    bank = i % 3
                    slot = i // 3
                    if nps == 3:
                        q = c % 3  # tile q -> bank q, never conflicts
                        j, h = c // 3, q
                    else:
                        q = 0  # unpacked: full-array tile, any bank ok
                        j, h = c, 0
                    src = T if q == 0 else Bc
                    nc.tensor.matmul(
                        op[:, bank : bank + 1, slot * cw : (slot + 1) * cw],
                        lhsT=src[
                            32 * q : 32 * q + 1, Pwt + j * 128 : Pwt + (j + 1) * 128
                        ],
                        rhs=src[32 * q : 32 * q + 1, h * cw : (h + 1) * cw],
                        start=True,
                        stop=True,
                        tile_position=(32 * q, 0),
                    )
                # LN rectangle: n%3==0 -> [3 banks, n//3 slots]; n<3 -> [n, 1]
                if n >= 3:
                    nb, m = 3, n // 3
                else:
                    nb, m = n, 1
                src = op[:, 0:nb, 0 : m * cw].rearrange("p b (i w) -> p b i w", i=m)
                dst = dummy_sb[:, 0:nb, 0 : m * cw].rearrange(
                    "p b (i w) -> p b i w", i=m
                )
                nc.scalar.activation(
                    dst,
                    src,
                    mybir.ActivationFunctionType.Ln,
                    bias=1.0,
                    scale=1.0,
                    accum_out=acc_sb[:, state["call"] : state["call"] + 1],
                )
                state["call"] += 1

            # chunk-to-block mapping sanity: chunk c covers neg block c//3
            # paired with pos third c%3; over c = 0..3*nch-1 this hits every
            # (j, h) exactly once.

            call_lo = 0
            stageAmm(0)
            stageAmm(1)
            stageScore(0)
            stageExp(0)
            stageBcast(0)
            for r in range(BPC):
                # keep DMA-gated phase-A/score work ahead of outer products
                # in the PE's in-order queue: outers for row r are only
                # ready ~1 row-period after row r's phase A ends.
                if r + 2 < BPC:
                    stageAmm(r + 2)
                if r + 1 < BPC:
                    stageScore(r + 1)
                    stageExp(r + 1)
                    stageBcast(r + 1)
                for gi in range(len(geo[r][4])):
                    stageOuterLn(r, gi)
                # ship this row's accumulator columns on the scalar queue
                nc.scalar.dma_start(
                    out_d[:, call_lo : state["call"]],
                    acc_sb[:, call_lo : state["call"]],
                )
                call_lo = state["call"]

    nc.compile()
    return nc, state["call"]


def kernel(keys, tgts, knn_tgts, mask, W1, b1, W2, b2, _profile=False):
    import ml_dtypes

    from concourse.bass_utils import run_bass_kernel_spmd

    keys = np.asarray(keys, dtype=np.float32)
    tgts = np.asarray(tgts)
    knn_tgts = np.asarray(knn_tgts)
    mask = np.asarray(mask).astype(bool)
    W1 = np.asarray(W1, dtype=np.float32)
    b1 = np.asarray(b1, dtype=np.float32)
    W2 = np.asarray(W2, dtype=np.float32)

    # ---- host-side label/permutation prep ----
    y = knn_tgts == tgts[:, None]
    pos = y & mask
    neg = (~y) & mask
    P = pos.sum(axis=1).astype(np.int64)
    N_ = neg.sum(axis=1).astype(np.int64)
    cnt = float((P * N_).sum())

    # stable order: positives, negatives, masked-out
    rank = np.where(pos, 0, np.where(neg, 1, 2)).astype(np.int8)
    order = np.argsort(rank, axis=1, kind="stable")  # [B, K]

    # deal rows sorted by P desc: rank i -> core i%8, slot i//8
    rows_by_p = np.argsort(-P, kind="stable")
    assign = rows_by_p.reshape(BPC, N_CORES)  # [slot, core] -> row id

    slot_params = []
    for r in range(BPC):
        ps = P[assign[r]]
        Pw = int(ps.max())
        nch = (K - int(ps.min()) + 127) // 128
        slot_params.append((Pw, nch))
    slot_params = tuple(slot_params)

    # permuted, transposed keys in fp8 DoubleRow layout: per row [128, 8K]
    keys_perm = np.take_along_axis(keys, order[:, :, None], axis=1)  # [B, K, D]
    kt = keys_perm.transpose(0, 2, 1).astype(ml_dtypes.float8_e4m3)  # [B, D, K]
    kt = np.ascontiguousarray(
        kt.reshape(B, 4, 2, 128, K).transpose(0, 3, 1, 2, 4).reshape(B, 128, 8 * K)
    )

    # scale W1 by 16 into fp8's sweet spot; fold 1/16 into W2 and 16 into b1
    hpad = 112
    ndc = 4
    w1s = (W1.T * 16.0).astype(np.float32)  # [D, H]
    w4 = np.zeros((ndc, 2, 128, hpad), dtype=np.float32)
    w4[:, :, :, :H] = w1s.reshape(ndc, 2, 128, H)
    w1t = np.ascontiguousarray(
        w4.transpose(2, 0, 1, 3).reshape(128, ndc * 2 * hpad)
    ).astype(ml_dtypes.float8_e4m3)
    w2v = (W2.reshape(H) / 16.0).astype(np.float32)
    w2a = np.zeros((H + 2, 2), dtype=np.float32)
    w2a[:H, 0] = w2v
    w2a[H, 0] = 1.0  # + npad row
    w2a[:H, 1] = -w2v
    w2a[H + 1, 1] = -1.0  # - ppad row
    w2a = w2a.astype(ml_dtypes.bfloat16)
    b1c = np.ascontiguousarray(b1.reshape(H, 1) * 16.0)

    # pad rows riding in hh partitions 100:102 (rows 4:6 of a 6-row block
    # whose rows 0:4 land on junk partitions 96:100):
    #   npad[j] = -KILL for j < P  (kills positives on the negative side)
    #   ppad[j] = +KILL for j >= P (kills non-positives on the positive side)
    kidx = np.arange(K)[None, :]
    pads = np.zeros((N_CORES, BPC, 6, K), dtype=np.float32)
    for r in range(BPC):
        pr = P[assign[r]][:, None]  # [cores, 1]
        pads[:, r, 4, :] = np.where(kidx < pr, -KILL, 0.0)
        pads[:, r, 5, :] = np.where(kidx < pr, 0.0, KILL)
    pads = pads.astype(ml_dtypes.bfloat16)

    key = slot_params
    if key not in _cache:
        _cache[key] = _build_program(slot_params)
    nc, ncalls = _cache[key]

    in_maps = []
    for c in range(N_CORES):
        in_maps.append(
            {
                "keys_t": np.ascontiguousarray(kt[assign[:, c]]),
                "w1t": w1t,
                "w2a": w2a,
                "b1c": b1c,
                "pads": np.ascontiguousarray(pads[c]),
            }
        )

    res = run_bass_kernel_spmd(
        nc, in_maps, list(range(N_CORES)), trace=bool(_profile)
    )
    total = 0.0
    for r in res.results:
        total += float(r["acc_out"].astype(np.float64).sum())
    if _profile:
        print(f"HW exec time: {res.exec_time_ns} ns")
        globals()["_last_results"] = res
    loss = np.float64(total) / np.float64(cnt)
    return np.array(loss, dtype=np.float32)


# revision 27
# speedup vs baseline: 1.2077x; 1.1517x over previous
"""Trainium2 Bass kernel for the kNN pairwise-ranking loss.

Math: with y = (knn_tgts == tgts), the masked pairwise BCE-with-logits loss
over differing-label pairs (j > i) collapses to

    loss = sum_b sum_{n in neg_b} sum_{p in pos_b} softplus(s_n - s_p) / cnt
    cnt  = sum_b |pos_b| * |neg_b|

because for a (pos, neg) pair the per-pair term is softplus(s_neg - s_pos)
regardless of orientation, and b2 cancels in score differences.

Host side: per batch row, permute keys so positives come first, then
negatives.  Rows are sorted by positive count and dealt to (core, slot) so
each slot's compile-time widths are tight.  Additive kill values (+-200 on
invalid lanes) make killed exponentials underflow to exactly 0.

Device (SPMD over 8 cores, 4 batch rows each):
  phase A (per row): h = relu(W1 @ keys^T + b1) via PE (fp8 DoubleRow, f32
                     psum).  hh is augmented with two DMA'd pad rows
                     (npad at p100, ppad at p101).
  phase B (per row): ONE score matmul stripe per sign into S row 0:
                     cols [0:Pwt) hold -(s_p + ppad) (lhsT -w2a col), cols
                     [Pwt : Pwt+128*nch) hold s_n + npad; ONE Exp pass
                     produces the combined strip T [1, Pwt+128*nch];
                     gpsimd partition_broadcast replicates T to all 128
                     partitions (Bc); rank-1 outer products run 3-way
                     row-packed on the PE (tile q = chunk%3 at quadrant
                     32q writes only PSUM bank q of the op tile) to form
                     e^{s_n - s_p}; Ln(1+x) with accum_out reduces each
                     op tile in one 4D-AP ACT pass.
  Keys DMA descriptors are posted in parallel on sync (keys) and scalar
  (consts) sequencers right at main start; per-row partial accumulator
  columns are DMA'd out on the scalar queue as soon as they are final.
Host gathers [128, ncalls] partial sums, reduces, divides by cnt.
"""

import numpy as np

B, K, D, H = 32, 1024, 1024, 100
N_CORES = 8
BPC = B // N_CORES  # batch rows per core
KILL = 200.0

_cache = {}
_act_patched = False


def _patch_act_tables():
    """Make Exp/Ln resolve to the single combined ACT table set."""
    global _act_patched
    if _act_patched:
        return
    import concourse.bacc as bacc
    import concourse.hw_specs as hw_specs
    import concourse.mybir as mybir

    orig = hw_specs.get_activation_tables
    combined = "natural_log_exp_and_others"

    def patched(arch):
        tabs = orig(arch)
        out = {}
        for name, funcs in tabs.items():
            f = set(funcs)
            if name != combined and combined in tabs:
                f.discard(mybir.ActivationFunctionType.Exp)
                f.discard(mybir.ActivationFunctionType.Ln)
            out[name] = f
        return out

    hw_specs.get_activation_tables = patched
    bacc.get_activation_tables = patched
    _act_patched = True


def _slot_geometry(Pw, nch):
    """Chunk geometry for one slot: unpacked rank-1 chunks, one neg
    128-block each, bank-major into 3-bank op tiles.  cw = Pw; chunks
    fill bank c//spb, slot c%spb, so each LN group is a clean rectangle
    ([nb, spb] full banks, or [1, n] within one bank for the tail)."""
    cw = Pw
    Pwt = Pw
    assert Pwt <= 512 and Pwt + 128 * nch <= 1408
    nchk = nch
    spb = 512 // cw  # chunk slots per PSUM bank
    cap = 3 * spb
    groups = []  # (c0, n) per LN call
    c0 = 0
    while c0 < nchk:
        n = min(cap, nchk - c0)
        if n % spb and n > spb:
            n -= n % spb  # keep full-bank rectangles; tail goes alone
        groups.append((c0, n))
        c0 += n
    return cw, Pwt, nchk, spb, groups


def _build_program(slot_params):
    """slot_params: tuple of (Pw, nch) per slot; nst = 1024 - 128*nch."""
    import concourse.bacc as bacc
    import concourse.mybir as mybir
    import concourse.tile as tile

    _patch_act_tables()

    f32 = mybir.dt.float32
    bf16 = mybir.dt.bfloat16
    fp8 = mybir.dt.float8e4
    ndc = 4  # contraction chunks (256 wide with DoubleRow)
    hpad = 112  # padded per-subrow weight stride (DoubleRow needs step%16==0)
    wchunk = 2 * hpad

    geo = [_slot_geometry(Pw, nch) for Pw, nch in slot_params]
    ncalls = sum(len(g[4]) for g in geo)

    nc = bacc.Bacc(
        "TRN2",
        target_bir_lowering=False,
        debug=False,
        enable_asserts=False,
        num_devices=N_CORES,
    )

    keys_d = nc.dram_tensor(
        "keys_t", [BPC, 128, ndc * 2 * K], fp8, kind="ExternalInput"
    ).ap()
    w1t_d = nc.dram_tensor("w1t", [128, ndc * wchunk], fp8, kind="ExternalInput").ap()
    w2a_d = nc.dram_tensor("w2a", [H + 2, 2], bf16, kind="ExternalInput").ap()
    b1_d = nc.dram_tensor("b1c", [H, 1], f32, kind="ExternalInput").ap()
    pads_d = nc.dram_tensor("pads", [BPC, 6, K], bf16, kind="ExternalInput").ap()
    out_d = nc.dram_tensor("acc_out", [128, ncalls], f32, kind="ExternalOutput").ap()

    with tile.TileContext(nc) as tc:
        with (
            tc.tile_pool(name="const", bufs=1) as cpool,
            tc.tile_pool(name="keys", bufs=2 * BPC + 2) as kpool,
            tc.tile_pool(name="h", bufs=BPC) as hpool,
            tc.tile_pool(name="T1", bufs=2) as t1pool,
            tc.tile_pool(name="hp", bufs=2, space="PSUM") as hp_pool,
            tc.tile_pool(name="op", bufs=2, space="PSUM") as op_pool,
        ):
            acc_sb = cpool.tile([128, ncalls], f32, tag="acc")
            dummy_sb = cpool.tile([128, 3, 512], bf16, tag="dummy")
            tmp_sb = cpool.tile([1, 2], f32, tag="tmp")

            # trigger the ACT table load as the first scalar-engine work;
            # reads uninitialized SBUF on purpose (no dep, result unused)
            nc.scalar.activation(
                tmp_sb[0:1, 1:2], tmp_sb[0:1, 0:1],
                mybir.ActivationFunctionType.Exp, scale=1.0,
            )

            # ---- DMA issue: consts on scalar, keys on sync, pads gpsimd ----
            w1t_sb = cpool.tile([128, ndc * wchunk], fp8, tag="w1t")
            nc.scalar.dma_start(w1t_sb[:], w1t_d[:])
            w2a_sb = cpool.tile([H + 2, 2], bf16, tag="w2a")
            nc.scalar.dma_start(w2a_sb[:], w2a_d[:])
            b1_sb = cpool.tile([H, 1], f32, tag="b1")
            nc.scalar.dma_start(b1_sb[:], b1_d[:])

            # row-0 keys as quarter-DMAs so the first matmul starts sooner;
            # rows 1-3 as halves, all on the sync queue in consumption order
            kq = []
            quads = []
            for i in range(4):
                kt = kpool.tile([128, 2 * K], fp8, tag="keys")
                nc.sync.dma_start(kt[:], keys_d[0, :, i * 2 * K : (i + 1) * 2 * K])
                quads.append((kt, 0))
            kq.append(quads)
            for r in range(1, BPC):
                quads = []
                for hf in range(2):
                    kt = kpool.tile([128, ndc * K], fp8, tag="keys")
                    nc.sync.dma_start(
                        kt[:], keys_d[r, :, hf * ndc * K : (hf + 1) * ndc * K]
                    )
                    quads += [(kt, 0), (kt, 2 * K)]
                kq.append(quads)

            # hh tiles: pad rows ride at partitions 100:102 via gpsimd SWDGE
            hhs = []
            for r in range(BPC):
                hh = hpool.tile([H + 2, K], bf16, tag="h")
                nc.gpsimd.dma_start(hh[96 : H + 2, :], pads_d[r, :, :])
                hhs.append(hh)

            scs = [None] * BPC
            Ts = [None] * BPC
            state = {"call": 0}

            def stageAmm(r):
                hh = hhs[r]
                hp0 = hp_pool.tile([H, 512], f32, tag="hpx")
                hp1 = hp_pool.tile([H, 512], f32, tag="hpx")
                hps = [hp0, hp1]
                for dc in range(ndc):
                    kt, c0 = kq[r][dc]
                    kt3 = kt[:, c0 : c0 + 2 * K].rearrange("p (i k) -> p i k", i=2)
                    w_sl = w1t_sb[:, dc * wchunk : (dc + 1) * wchunk].rearrange(
                        "p (i m) -> p i m", i=2
                    )[:, :, 0:H]
                    for kh in range(2):
                        nc.tensor.matmul(
                            hps[kh][:, :],
                            lhsT=w_sl,
                            rhs=kt3[:, :, kh * 512 : (kh + 1) * 512],
                            start=(dc == 0),
                            stop=(dc == ndc - 1),
                            perf_mode=mybir.MatmulPerfMode.DoubleRow,
                        )
                for kh, eng in ((0, nc.vector), (1, nc.vector)):
                    eng.tensor_scalar(
                        hh[0:H, kh * 512 : (kh + 1) * 512],
                        hps[kh][:, :],
                        b1_sb[:],
                        0.0,
                        op0=mybir.AluOpType.add,
                        op1=mybir.AluOpType.max,
                    )

            def stageScore(r):
                """Scores into S row 0: [0:Pwt) = -(s_p+ppad), then
                [Pwt : Pwt+128*nch) = s_n + npad for keys nst:K."""
                Pw, nch = slot_params[r]
                cw, Pwt, nchk, spb, groups = geo[r]
                nst = K - 128 * nch
                hh = hhs[r]
                S = op_pool.tile([64, 3 * 512], f32, tag="op")
                scs[r] = S
                # pos: needs only the kh0 half of relu (Pwt <= 512)
                nc.tensor.matmul(
                    S[0:1, 0:Pwt],
                    lhsT=w2a_sb[:, 1:2],
                    rhs=hh[:, 0:Pwt],
                    start=True,
                    stop=True,
                )
                # neg: hh cols nst:K -> S cols Pwt:Pwt+128*nch, split on
                # both hh kh halves (512) and S psum bank boundaries (512)
                segs = []
                c = nst
                while c < K:
                    hh_end = 512 if c < 512 else K
                    s_col = Pwt + (c - nst)
                    s_end = c + (512 - s_col % 512) if (s_col % 512) else c + 512
                    e = min(hh_end, s_end, K)
                    segs.append((c, e))
                    c = e
                for c, e in segs:
                    nc.tensor.matmul(
                        S[0:1, Pwt + (c - nst) : Pwt + (e - nst)],
                        lhsT=w2a_sb[:, 0:1],
                        rhs=hh[:, c:e],
                        start=True,
                        stop=True,
                    )

            def stageExp(r):
                Pw, nch = slot_params[r]
                cw, Pwt, nchk, spb, groups = geo[r]
                L = Pwt + 128 * nch
                T = t1pool.tile([1, 1408], bf16, tag="T1")
                Ts[r] = T
                nc.scalar.activation(
                    T[0:1, 0:L],
                    scs[r][0:1, 0:L],
                    mybir.ActivationFunctionType.Exp,
                    scale=1.0,
                )

            def stageOuterLn(r, gi):
                """Emit one op tile's chunks (rank-1 matmuls) + 1 Ln."""
                Pw, nch = slot_params[r]
                cw, Pwt, nchk, spb, groups = geo[r]
                c0, n = groups[gi]
                T = Ts[r]
                op = op_pool.tile([128, 3, 512], f32, tag="op")
                for i in range(n):
                    c = c0 + i  # neg 128-block index
                    bank, slot = i // spb, i % spb
                    nc.tensor.matmul(
                        op[:, bank : bank + 1, slot * cw : (slot + 1) * cw],
                        lhsT=T[0:1, Pwt + c * 128 : Pwt + (c + 1) * 128],
                        rhs=T[0:1, 0:Pwt],
                        start=True,
                        stop=True,
                    )
                # LN rectangle: full banks [nb, spb] or one-bank tail [1, n]
                if n >= spb:
                    nb, m = n // spb, spb
                else:
                    nb, m = 1, n
                src = op[:, 0:nb, 0 : m * cw].rearrange("p b (i w) -> p b i w", i=m)
                dst = dummy_sb[:, 0:nb, 0 : m * cw].rearrange(
                    "p b (i w) -> p b i w", i=m
                )
                nc.scalar.activation(
                    dst,
                    src,
                    mybir.ActivationFunctionType.Ln,
                    bias=1.0,
                    scale=1.0,
                    accum_out=acc_sb[:, state["call"] : state["call"] + 1],
                )
                state["call"] += 1

            # chunk-to-block mapping sanity: chunk c covers neg block c//3
            # paired with pos third c%3; over c = 0..3*nch-1 this hits every
            # (j, h) exactly once.

            call_lo = 0
            stageAmm(0)
            stageAmm(1)
            stageScore(0)
            stageExp(0)
            for r in range(BPC):
                ngroups = len(geo[r][4])
                if r == 0:
                    # start-up: row-1's scores aren't ready yet; issuing
                    # exp(1) first would block ready Ln(0) work on ACT
                    stageOuterLn(0, 0)
                    stageAmm(2)
                    stageScore(1)
                    stageExp(1)
                    for gi in range(1, ngroups):
                        stageOuterLn(0, gi)
                else:
                    if r + 2 < BPC:
                        stageAmm(r + 2)
                    if r + 1 < BPC:
                        stageScore(r + 1)
                        stageExp(r + 1)
                    for gi in range(ngroups):
                        stageOuterLn(r, gi)
                # ship this row's accumulator columns on the scalar queue
                nc.scalar.dma_start(
                    out_d[:, call_lo : state["call"]],
                    acc_sb[:, call_lo : state["call"]],
                )
                call_lo = state["call"]

    nc.compile()
    return nc, state["call"]


def kernel(keys, tgts, knn_tgts, mask, W1, b1, W2, b2, _profile=False):
    import ml_dtypes

    from concourse.bass_utils import run_bass_kernel_spmd

    keys = np.asarray(keys, dtype=np.float32)
    tgts = np.asarray(tgts)
    knn_tgts = np.asarray(knn_tgts)
    mask = np.asarray(mask).astype(bool)
    W1 = np.asarray(W1, dtype=np.float32)
    b1 = np.asarray(b1, dtype=np.float32)
    W2 = np.asarray(W2, dtype=np.float32)

    # ---- host-side label/permutation prep ----
    y = knn_tgts == tgts[:, None]
    pos = y & mask
    neg = (~y) & mask
    P = pos.sum(axis=1).astype(np.int64)
    N_ = neg.sum(axis=1).astype(np.int64)
    cnt = float((P * N_).sum())

    # stable order: positives, negatives, masked-out
    rank = np.where(pos, 0, np.where(neg, 1, 2)).astype(np.int8)
    order = np.argsort(rank, axis=1, kind="stable")  # [B, K]

    # deal rows sorted by P desc: rank i -> core i%8, slot i//8
    rows_by_p = np.argsort(-P, kind="stable")
    assign = rows_by_p.reshape(BPC, N_CORES)  # [slot, core] -> row id

    slot_params = []
    for r in range(BPC):
        ps = P[assign[r]]
        Pw = int(ps.max())
        nch = (K - int(ps.min()) + 127) // 128
        slot_params.append((Pw, nch))
    slot_params = tuple(slot_params)

    # permuted, transposed keys in fp8 DoubleRow layout: per row [128, 8K]
    keys_perm = np.take_along_axis(keys, order[:, :, None], axis=1)  # [B, K, D]
    kt = keys_perm.transpose(0, 2, 1).astype(ml_dtypes.float8_e4m3)  # [B, D, K]
    kt = np.ascontiguousarray(
        kt.reshape(B, 4, 2, 128, K).transpose(0, 3, 1, 2, 4).reshape(B, 128, 8 * K)
    )

    # scale W1 by 16 into fp8's sweet spot; fold 1/16 into W2 and 16 into b1
    hpad = 112
    ndc = 4
    w1s = (W1.T * 16.0).astype(np.float32)  # [D, H]
    w4 = np.zeros((ndc, 2, 128, hpad), dtype=np.float32)
    w4[:, :, :, :H] = w1s.reshape(ndc, 2, 128, H)
    w1t = np.ascontiguousarray(
        w4.transpose(2, 0, 1, 3).reshape(128, ndc * 2 * hpad)
    ).astype(ml_dtypes.float8_e4m3)
    w2v = (W2.reshape(H) / 16.0).astype(np.float32)
    w2a = np.zeros((H + 2, 2), dtype=np.float32)
    w2a[:H, 0] = w2v
    w2a[H, 0] = 1.0  # + npad row
    w2a[:H, 1] = -w2v
    w2a[H + 1, 1] = -1.0  # - ppad row
    w2a = w2a.astype(ml_dtypes.bfloat16)
    b1c = np.ascontiguousarray(b1.reshape(H, 1) * 16.0)

    # pad rows riding in hh partitions 100:102 (rows 4:6 of a 6-row block
    # whose rows 0:4 land on junk partitions 96:100):
    #   npad[j] = -KILL for j < P  (kills positives on the negative side)
    #   ppad[j] = +KILL for j >= P (kills non-positives on the positive side)
    kidx = np.arange(K)[None, :]
    pads = np.zeros((N_CORES, BPC, 6, K), dtype=np.float32)
    for r in range(BPC):
        pr = P[assign[r]][:, None]  # [cores, 1]
        pads[:, r, 4, :] = np.where(kidx < pr, -KILL, 0.0)
        pads[:, r, 5, :] = np.where(kidx < pr, 0.0, KILL)
    pads = pads.astype(ml_dtypes.bfloat16)

    key = slot_params
    if key not in _cache:
        _cache[key] = _build_program(slot_params)
    nc, ncalls = _cache[key]

    in_maps = []
    for c in range(N_CORES):
        in_maps.append(
            {
                "keys_t": np.ascontiguousarray(kt[assign[:, c]]),
                "w1t": w1t,
                "w2a": w2a,
                "b1c": b1c,
                "pads": np.ascontiguousarray(pads[c]),
            }
        )

    res = run_bass_kernel_spmd(
        nc, in_maps, list(range(N_CORES)), trace=bool(_profile)
    )
    total = 0.0
    for r in res.results:
        total += float(r["acc_out"].astype(np.float64).sum())
    if _profile:
        print(f"HW exec time: {res.exec_time_ns} ns")
        globals()["_last_results"] = res
    loss = np.float64(total) / np.float64(cnt)
    return np.array(loss, dtype=np.float32)
